# revision 37
# baseline (speedup 1.0000x reference)
"""Trainium2 Bass kernel for nn_BRC_17179869451 (BRC-style RNN).

  xz/xr/xh = x @ {kz,kr,kh}   (three [B*T,D]x[D,H] GEMMs)
  scan over T:
      r = tanh(xr_t + h*mr + br) + 1
      z = sigmoid(xz_t + h*mz + bz)
      h = z*h + (1-z)*tanh(xh_t + r*h)

Sharding (8 cores = 4 time-segments x 2 batch-halves): the BRC forget
gate makes h_t depend only weakly on the distant past, so each core
computes a 128-step time segment for its 32-batch half, preceded by a
W=48-step redundant warmup from h=0 (validated offline: rel err ~2e-4
in fp64, ~2e-3 with the fp16 pipeline below).  Segment 0 zero-pads its
warmup input, which keeps h exactly 0.

Everything on-device runs fp16 (validated rel err ~1.9e-3 end to end):
fp16 GEMMs (1 PE pass instead of 4 for fp32), fp16 scan ops (DVE 2x/4x
perf modes), fp16 output staged via the xbar DMA-transpose and upcast
to fp32 on the host.

Per-step math (fast path mz=mr=1; hh = h+1 shifted state, hm = h):
  s = sigmoid(2*(xr-1 + hh))            r = 2s
  q = sigmoid(4*(hm*s + xh/2))          tanh(xh + r*h) = 2q-1
  z = sigmoid(xz + hm)
  hh' = 2q(1-z) + hh*z ;  ys = hm' = hh' - 1
as engine ops (V=DVE, A=ACT, G=gpsimd), gates pre-scaled in epilogue:
  chain: v=TT(q,U2) -> a+=TT(v,hz1x) -> s=ACT(a,sc2) -> sh=TT(hm,s)
         -> e3=TT(sh,XHH) -> q=ACT(e3,sc4)
  off:   hh+=TT(v,hz1); hm+=TS(hh+,-1); c+=G.TT(XZ0,hm+); z=ACT(c);
         U2=TS(z,-2,+2); hz1=TT(hh,z); hz1x=TT(hz1,XR1[t+1])
Layout per core: state [128 x 256]: partition h_a = h mod 128, free
(hb = h div 128 [8], b [32]).  Output: per 16-step chunk the hm ring
[128, (t,j,u)] is xbar-transposed to [u, (t,j), h_a] and DMA'd to
ys[b,t,h] in 256B runs.
"""

import os
import numpy as np

B, T, D, H = 64, 512, 512, 1024
NCORES = 8
ST = 4                    # time segments
SB = 2                    # batch shards
BC = B // SB              # 32 batches per core
SEG = T // ST             # 128 output steps per core
W = 32                    # warmup steps
N = SEG + W               # 176 steps computed per core
TC = 32                   # steps per chunk
NCH = N // TC             # 5 chunks
OC0 = W // TC             # first output chunk (1)
HB = H // 128             # 8 h-blocks
P = HB * BC               # 256 = free size of scan state
KT = D // 128             # 4 k-tiles
CB = TC * BC              # 512 matmul cols per chunk

_cache = {}


def _apply_tile_drain_patch():
    """Spread end-of-kernel sem waits over single-wait sync nops: walrus
    CoreV3 codegen rejects the stock Tile exit Drain that carries one wait
    per logical proc ("Too many sync wait commands")."""
    import concourse.tile as tile_mod

    if getattr(tile_mod.TileContext, "_drain_patched", False):
        return

    def _patched(self, tick_clock, wait_clock):
        from concourse.vector_clock import ScopedClock

        vclock = tick_clock.global_clock
        pend = [(p, vclock[p]) for p in range(len(vclock)) if vclock[p] > 0]
        for proc, tick in pend:
            sub = ScopedClock()
            sub.require_at_least(None, proc, tick)
            nop_inst = self.nc.sync.nop(nofuse=True)
            wait_clock.add_sem_waits(nop_inst.ins, sub)
        self.nc.sync.drain()
        self.nc.all_engine_barrier()
        assert self.sems is not None
        popped = self.nc._tile_sem_poison_stack.pop()
        assert popped is self._sem_poison
        self.nc.clear_and_free_semaphores(list(self.sems.allocated().values()))
        self.nc.all_engine_barrier()

    tile_mod.TileContext._drain_and_barrier = _patched
    tile_mod.TileContext._drain_patched = True


def _legalize_sync_waits(nc, max_waits: int = 1):
    """walrus codegen here rejects instructions with >1 sem wait ("Too many
    sync wait commands"); hoist extra waits onto same-engine NoOps."""
    import concourse.mybir as mybir

    n = 0
    for f in nc.m.functions:
        for bb in f.blocks:
            out = []
            for ins in bb.instructions:
                si = ins.sync_info
                if si is not None and si.on_wait and len(si.on_wait) > max_waits:
                    waits = list(si.on_wait)
                    for w in waits[:-max_waits]:
                        n += 1
                        nop = mybir.InstNoOp(
                            name=f"waitnop_{n}", engine=ins.engine)
                        nop.sync_info = mybir.SyncInfo(
                            on_wait=[w], on_update=[])
                        out.append(nop)
                    si.on_wait = waits[-max_waits:]
                out.append(ins)
            bb.instructions = out


def _build(fast: bool):
    import concourse.bass as bass
    import concourse.mybir as mybir
    from concourse.tile import TileContext

    _apply_tile_drain_patch()

    fp16 = mybir.dt.float16
    fp32 = mybir.dt.float32
    AF = mybir.ActivationFunctionType
    OP = mybir.AluOpType

    nc = bass.Bass()
    xT_d = nc.dram_tensor("xT", [D, N, BC], fp16, kind="ExternalInput")
    kz_d = nc.dram_tensor("kz", [D, H], fp16, kind="ExternalInput")
    kr_d = nc.dram_tensor("kr", [D, H], fp16, kind="ExternalInput")
    kh_d = nc.dram_tensor("kh", [D, H], fp16, kind="ExternalInput")
    # per-hb epilogue bias columns [128, HB]: fast: XR bias = br-mr (=-1)
    brv_d = nc.dram_tensor("brv", [128, HB], fp32, kind="ExternalInput")
    if not fast:
        bzv_d = nc.dram_tensor("bzv", [128, HB], fp32, kind="ExternalInput")
        mrt_d = nc.dram_tensor("mrt", [128, P], fp16, kind="ExternalInput")
        mzt_d = nc.dram_tensor("mzt", [128, P], fp16, kind="ExternalInput")
    # ys stored [l, b, t, j, c] (h = (j*4+l)*128+c) so the post-transpose
    # chunk DMA is perfectly linear; host reassembles to [b, t, h].
    ys_d = nc.dram_tensor("ys", [HB // 2, BC, SEG, 2, 128], fp16,
                          kind="ExternalOutput")

    with TileContext(nc) as tc:
        with (
            tc.tile_pool(name="const", bufs=1) as cpool,
            tc.tile_pool(name="xk", bufs=2) as xkpool,
            tc.tile_pool(name="gates", bufs=2) as gpool,
            tc.tile_pool(name="scan", bufs=3) as spool,
            tc.tile_pool(name="ring", bufs=2) as rpool,
            tc.tile_pool(name="stg", bufs=1) as stpool,
            tc.tile_pool(name="psmm", bufs=3, space="PSUM") as pspool,
            tc.tile_pool(name="psmmh", bufs=2, space="PSUM") as pspoolh,
        ):
            # ---- weights / constants ----
            w_sb = {}
            for name, wd in (("z", kz_d), ("r", kr_d), ("h", kh_d)):
                wt = cpool.tile([128, KT * H], fp16, tag=f"w{name}")
                nc.sync.dma_start(
                    out=wt.rearrange("p (k h) -> p k h", k=KT),
                    in_=wd.rearrange("(k p) h -> p k h", p=128))
                for k in range(KT):
                    w_sb[(name, k)] = wt[:, k * H:(k + 1) * H]
            brv = cpool.tile([128, HB], fp32, tag="brv")
            nc.sync.dma_start(out=brv, in_=brv_d[:, :])
            if not fast:
                bzv = cpool.tile([128, HB], fp32, tag="bzv")
                nc.sync.dma_start(out=bzv, in_=bzv_d[:, :])
                mrt = cpool.tile([128, P], fp16, tag="mrt")
                nc.sync.dma_start(out=mrt, in_=mrt_d[:, :])
                mzt = cpool.tile([128, P], fp16, tag="mzt")
                nc.sync.dma_start(out=mzt, in_=mzt_d[:, :])

            hh0 = cpool.tile([128, P], fp16, tag="hh0")
            nc.vector.memset(hh0, 1.0)   # hh = h+1, h0 = 0
            hm0 = cpool.tile([128, P], fp16, tag="hm0")
            nc.vector.memset(hm0, 0.0)

            import bass_rust as _br

            _last = {}

            def _pin(eng, bi):
                # Pin each engine's stream to emission order; prevents
                # scheduler priority inversions (engines execute in-order).
                if eng in _last:
                    _br.add_dep_helper(bi.ins, _last[eng].ins, sync=False,
                                       reason=f"{eng} emission order")
                _last[eng] = bi
                return bi

            def vop(bi):
                return _pin("v", bi)

            def aop(bi):
                return _pin("a", bi)

            def gop(bi):
                return _pin("g", bi)

            def pe(bi):
                return _pin("pe", bi)

            # ---- GEMM pieces per chunk ----
            gates = {}   # ci -> (XR, XZ, XH) sbuf tiles [128, TC*P] fp16

            def make_pieces(ci):
                """Returns (loads, vps, aps, gps): closures for chunk ci's
                x loads and per-(gate,hb) matmul+epilogue groups, keyed by
                the epilogue engine."""
                XR = gpool.tile([128, TC * P], fp16, tag="XR", name=f"XR{ci}")
                XZ = gpool.tile([128, TC * P], fp16, tag="XZ", name=f"XZ{ci}")
                XH = gpool.tile([128, TC * P], fp16, tag="XH", name=f"XH{ci}")
                gates[ci] = (XR, XZ, XH)
                xk = [xkpool.tile([128, CB], fp16, tag=f"xk{k}",
                                  name=f"xk{k}_{ci}") for k in range(KT)]

                def load(k, xk=xk, ci=ci):
                    nc.sync.dma_start(
                        out=xk[k],
                        in_=xT_d[k * 128:(k + 1) * 128,
                                 ci * TC:(ci + 1) * TC, :])
                loads = [lambda k=k: load(k) for k in range(KT)]

                def mmgroup(g, hb, dest, ci=ci, xk=xk, lo=0, hi=TC):
                    nt = hi - lo
                    pool = pspool if nt == TC else pspoolh
                    ps = pool.tile([128, nt * BC], fp32,
                                   tag=("mm" if nt == TC else "mmh"),
                                   name=f"mm{ci}_{g}{hb}_{lo}")
                    # matmul free size is capped at 512 fp32 (one PSUM
                    # bank); tile wider groups into bank-sized column sets
                    for c0 in range(0, nt * BC, 512):
                        c1 = min(c0 + 512, nt * BC)
                        for k in range(KT):
                            pe(nc.tensor.matmul(
                                out=ps[:, c0:c1],
                                lhsT=w_sb[(g, k)][:, hb * 128:(hb + 1) * 128],
                                rhs=xk[k][:, lo * BC + c0:lo * BC + c1],
                                start=(k == 0), stop=(k == KT - 1)))
                    dst = dest.rearrange(
                        "p (t hb b) -> p t hb b", t=TC, hb=HB)[:, lo:hi, hb, :]
                    ps3 = ps.rearrange("p (t b) -> p t b", t=nt)
                    # all epilogues on ACT: DVE is the throughput-bound
                    # engine, ACT has ~1us/step of idle after its sigmoids
                    if g == "r":      # XR: xr + (br - mr)
                        aop(nc.scalar.activation(
                            out=dst, in_=ps3, func=AF.Identity,
                            bias=brv[:, hb:hb + 1], scale=1.0))
                    elif g == "h":    # XH: xh / 2
                        aop(nc.scalar.activation(
                            out=dst, in_=ps3, func=AF.Identity,
                            bias=0.0, scale=0.5))
                    else:             # XZ: xz (+ bz)
                        aop(nc.scalar.activation(
                            out=dst, in_=ps3, func=AF.Identity,
                            bias=(0.0 if fast else bzv[:, hb:hb + 1]),
                            scale=1.0))

                eps = []
                if ci == 0:
                    # quarter chunk 0's groups so the scan can start after
                    # only the first 8 steps' gates are ready
                    for qi in range(4):
                        lo, hi = qi * TC // 4, (qi + 1) * TC // 4
                        for hb in range(HB):
                            for g, dd in (("r", XR), ("z", XZ), ("h", XH)):
                                eps.append(
                                    lambda g=g, hb=hb, dd=dd, lo=lo, hi=hi:
                                    mmgroup(g, hb, dd, lo=lo, hi=hi))
                else:
                    for hb in range(HB):
                        eps.append(lambda hb=hb: mmgroup("r", hb, XR))
                        eps.append(lambda hb=hb: mmgroup("z", hb, XZ))
                        eps.append(lambda hb=hb: mmgroup("h", hb, XH))
                return [loads, eps]

            def emit_output(ci, ring):
                """xbar-transpose chunk ci's hm ring and DMA to ys."""
                stg = stpool.tile([128, TC * P], fp16, tag="stg",
                                  name=f"stg{ci}")
                nc.sync.dma_start_transpose(
                    out=stg.rearrange("p (g m) -> p g m", m=128),
                    in_=ring.rearrange("p (g u) -> p g u", u=128))
                ot0 = ci * TC - W
                dst = ys_d[:, :, ot0:ot0 + TC, :, :].rearrange(
                    "l b t j c -> (l b) t j c")
                nc.sync.dma_start(
                    out=dst,
                    in_=stg.rearrange("p (t j c) -> p t j c", t=TC, j=2))

            # ---- emit: prime chunk 0, then scan with 1-chunk-ahead GEMM ----
            for grp in make_pieces(0):
                for p_ in grp:
                    p_()

            hh, hm = hh0, hm0

            def s_tile(tag, i):
                return spool.tile([128, P], fp16, tag=tag, name=f"{tag}_{i}")

            def gate_col(gt, t):
                return gt[:, t * P:(t + 1) * P]

            # a_0 / c_0 (and general-path m*h temps)
            XR, XZ, XH = gates[0]
            a_t = s_tile("a", 0)
            c_t = s_tile("c", 0)
            if fast:
                vop(nc.vector.tensor_tensor(a_t, gate_col(XR, 0), hh, OP.add))
                gop(nc.gpsimd.tensor_tensor(c_t, gate_col(XZ, 0), hm, OP.add))
            else:
                t1 = s_tile("t1", 0)
                vop(nc.vector.tensor_tensor(t1, mrt, hm, OP.mult))
                vop(nc.vector.tensor_tensor(a_t, t1, gate_col(XR, 0), OP.add))
                t2 = s_tile("t2", 0)
                vop(nc.vector.tensor_tensor(t2, mzt, hm, OP.mult))
                vop(nc.vector.tensor_tensor(c_t, t2, gate_col(XZ, 0), OP.add))

            for ci in range(NCH):
                XR, XZ, XH = gates[ci]
                nxt = make_pieces(ci + 1) if ci + 1 < NCH else [[], []]
                loads, eps = nxt
                ring = (rpool.tile([128, TC * P], fp16, tag="ring",
                                   name=f"ring{ci}") if ci >= OC0 else None)
                # all of chunk ci+2's x loads must be emitted before any of
                # its matmul groups (else early groups read stale x tiles)
                while loads:
                    loads.pop(0)()
                for t in range(TC):
                    i = ci * TC + t
                    last = (i == N - 1)
                    # chain front: s, sh, e3, q  (a_t from previous tail)
                    s_ = s_tile("s", i)
                    aop(nc.scalar.activation(s_, a_t, AF.Sigmoid, scale=2.0))
                    sh = s_tile("sh", i)
                    vop(nc.vector.tensor_tensor(sh, hm, s_, OP.mult))
                    e3 = s_tile("e3", i)
                    vop(nc.vector.tensor_tensor(e3, sh, gate_col(XH, t),
                                                OP.add))
                    z_ = s_tile("z", i)
                    aop(nc.scalar.activation(z_, c_t, AF.Sigmoid))
                    q_ = s_tile("q", i)
                    aop(nc.scalar.activation(q_, e3, AF.Sigmoid, scale=4.0))
                    # 24 epilogue groups over 32 steps: 3 per 4 steps, all
                    # in ACT's post-q idle window
                    if t % 4 != 3 and eps:
                        eps.pop(0)()
                    # off-chain tail
                    U2 = s_tile("U2", i)
                    vop(nc.vector.tensor_scalar(
                        out=U2, in0=z_, scalar1=-2.0, scalar2=2.0,
                        op0=OP.mult, op1=OP.add))
                    hz1 = s_tile("hz1", i)
                    vop(nc.vector.tensor_tensor(hz1, hh, z_, OP.mult))
                    if fast and not last:
                        hz1x = s_tile("hz1x", i)
                        XRn = gates[ci + 1][0] if t == TC - 1 else XR
                        vop(nc.vector.tensor_tensor(
                            hz1x, hz1, gate_col(XRn, (t + 1) % TC), OP.add))
                    v_ = s_tile("vv", i)
                    vop(nc.vector.tensor_tensor(v_, q_, U2, OP.mult))
                    # chain-critical a' goes right after v; hh'/hm' have
                    # ~half a step of slack before their first consumers
                    if not last and fast:
                        a_t = s_tile("a", i + 1)
                        vop(nc.vector.tensor_tensor(a_t, v_, hz1x, OP.add))
                    hh_n = s_tile("hh", i)
                    vop(nc.vector.tensor_tensor(hh_n, v_, hz1, OP.add))
                    hm_n = (ring[:, t * P:(t + 1) * P] if ring is not None
                            else s_tile("hm", i))
                    vop(nc.vector.tensor_scalar(
                        out=hm_n, in0=hh_n, scalar1=-1.0, scalar2=None,
                        op0=OP.add))
                    if not last:
                        c_t = s_tile("c", i + 1)
                        if fast:
                            XZn = gates[ci + 1][1] if t == TC - 1 else XZ
                            vop(nc.vector.tensor_tensor(
                                c_t, gate_col(XZn, (t + 1) % TC), hm_n,
                                OP.add))
                        else:
                            a_t = s_tile("a", i + 1)
                            XRn = gates[ci + 1][0] if t == TC - 1 else XR
                            XZn = gates[ci + 1][1] if t == TC - 1 else XZ
                            t1 = s_tile("t1", i + 1)
                            vop(nc.vector.tensor_tensor(t1, mrt, hm_n,
                                                        OP.mult))
                            vop(nc.vector.tensor_tensor(
                                a_t, t1, gate_col(XRn, (t + 1) % TC), OP.add))
                            t2 = s_tile("t2", i + 1)
                            gop(nc.gpsimd.tensor_tensor(t2, mzt, hm_n,
                                                        OP.mult))
                            gop(nc.gpsimd.tensor_tensor(
                                c_t, t2, gate_col(XZn, (t + 1) % TC), OP.add))
                    hh = hh_n
                    hm = hm_n
                # drain leftover pieces, then output the chunk
                for grp in (loads, eps):
                    while grp:
                        grp.pop(0)()
                if ring is not None:
                    emit_output(ci, ring)

    _legalize_sync_waits(nc)
    return nc


def _get_nc(fast: bool):
    if fast not in _cache:
        _cache[fast] = _build(fast)
    return _cache[fast]


LAST_RESULT = None


def kernel(**inputs):
    global LAST_RESULT
    from concourse.bass_utils import run_bass_kernel_spmd

    x = np.asarray(inputs["x"], dtype=np.float32)
    kz = np.asarray(inputs["kz"], dtype=np.float32)
    kr = np.asarray(inputs["kr"], dtype=np.float32)
    kh = np.asarray(inputs["kh"], dtype=np.float32)
    mz = np.asarray(inputs["mz"], dtype=np.float32)
    mr = np.asarray(inputs["mr"], dtype=np.float32)
    br = np.asarray(inputs["br"], dtype=np.float32)
    bz = np.asarray(inputs["bz"], dtype=np.float32)
    assert x.shape == (B, T, D) and kz.shape == (D, H)

    fast = bool(np.all(mz == 1.0) and np.all(mr == 1.0))
    nc = _get_nc(fast)

    def pvec(v):  # [H] -> [128, HB] with [h_a, h_b]
        return np.ascontiguousarray(v.reshape(HB, 128).T)

    def ptile(v):  # [H] -> [128, (hb, b)] fp16, replicated over b
        t = v.reshape(HB, 128).T
        return np.ascontiguousarray(
            np.repeat(t[:, :, None], BC, axis=2).reshape(128, P)
        ).astype(np.float16)

    base = {
        "kz": np.ascontiguousarray(kz).astype(np.float16),
        "kr": np.ascontiguousarray(kr).astype(np.float16),
        "kh": np.ascontiguousarray(kh).astype(np.float16),
        "brv": pvec((br - mr) if fast else br).astype(np.float32),
    }
    if not fast:
        base["bzv"] = pvec(bz).astype(np.float32)
        base["mrt"] = ptile(mr)
        base["mzt"] = ptile(mz)

    x16 = x.astype(np.float16)
    in_maps = []
    for i in range(NCORES):
        i_t, i_b = i // SB, i % SB
        t0 = i_t * SEG
        bs = slice(i_b * BC, (i_b + 1) * BC)
        xc = np.zeros((BC, N, D), np.float16)
        src = x16[bs, max(0, t0 - W):t0 + SEG]
        xc[:, N - src.shape[1]:, :] = src
        xTc = np.ascontiguousarray(xc.transpose(2, 1, 0))
        in_maps.append(dict(base, xT=xTc))

    trace = bool(int(os.environ.get("KERNEL_TRACE", "0")))
    res = run_bass_kernel_spmd(nc, in_maps, list(range(NCORES)), trace=trace)
    LAST_RESULT = res
    ys = np.empty((B, T, H), np.float32)
    for i in range(NCORES):
        i_t, i_b = i // SB, i % SB
        yc = res.results[i]["ys"].astype(np.float32)  # [l, b, t, j, c]
        ys[i_b * BC:(i_b + 1) * BC, i_t * SEG:(i_t + 1) * SEG, :] = (
            yc.transpose(1, 2, 3, 0, 4).reshape(BC, SEG, H))
    return ys


# revision 38
# speedup vs baseline: 1.0322x; 1.0322x over previous
"""Trainium2 Bass kernel for nn_BRC_17179869451 (BRC-style RNN).

  xz/xr/xh = x @ {kz,kr,kh}   (three [B*T,D]x[D,H] GEMMs)
  scan over T:
      r = tanh(xr_t + h*mr + br) + 1
      z = sigmoid(xz_t + h*mz + bz)
      h = z*h + (1-z)*tanh(xh_t + r*h)

Sharding (8 cores = 4 time-segments x 2 batch-halves): the BRC forget
gate makes h_t depend only weakly on the distant past, so each core
computes a 128-step time segment for its 32-batch half, preceded by a
W=48-step redundant warmup from h=0 (validated offline: rel err ~2e-4
in fp64, ~2e-3 with the fp16 pipeline below).  Segment 0 zero-pads its
warmup input, which keeps h exactly 0.

Everything on-device runs fp16 (validated rel err ~1.9e-3 end to end):
fp16 GEMMs (1 PE pass instead of 4 for fp32), fp16 scan ops (DVE 2x/4x
perf modes), fp16 output staged via the xbar DMA-transpose and upcast
to fp32 on the host.

Per-step math (fast path mz=mr=1; hh = h+1 shifted state, hm = h):
  s = sigmoid(2*(xr-1 + hh))            r = 2s
  q = sigmoid(4*(hm*s + xh/2))          tanh(xh + r*h) = 2q-1
  z = sigmoid(xz + hm)
  hh' = 2q(1-z) + hh*z ;  ys = hm' = hh' - 1
as engine ops (V=DVE, A=ACT, G=gpsimd), gates pre-scaled in epilogue:
  chain: v=TT(q,U2) -> a+=TT(v,hz1x) -> s=ACT(a,sc2) -> sh=TT(hm,s)
         -> e3=TT(sh,XHH) -> q=ACT(e3,sc4)
  off:   hh+=TT(v,hz1); hm+=TS(hh+,-1); c+=G.TT(XZ0,hm+); z=ACT(c);
         U2=TS(z,-2,+2); hz1=TT(hh,z); hz1x=TT(hz1,XR1[t+1])
Layout per core: state [128 x 256]: partition h_a = h mod 128, free
(hb = h div 128 [8], b [32]).  Output: per 16-step chunk the hm ring
[128, (t,j,u)] is xbar-transposed to [u, (t,j), h_a] and DMA'd to
ys[b,t,h] in 256B runs.
"""

import os
import numpy as np

B, T, D, H = 64, 512, 512, 1024
NCORES = 8
ST = 4                    # time segments
SB = 2                    # batch shards
BC = B // SB              # 32 batches per core
SEG = T // ST             # 128 output steps per core
W = 32                    # warmup steps
N = SEG + W               # 176 steps computed per core
TC = 16                   # steps per chunk
NCH = N // TC             # 11 chunks
OC0 = W // TC             # first output chunk (3)
HB = H // 128             # 8 h-blocks
P = HB * BC               # 256 = free size of scan state
KT = D // 128             # 4 k-tiles
CB = TC * BC              # 512 matmul cols per chunk

_cache = {}


def _apply_tile_drain_patch():
    """Spread end-of-kernel sem waits over single-wait sync nops: walrus
    CoreV3 codegen rejects the stock Tile exit Drain that carries one wait
    per logical proc ("Too many sync wait commands")."""
    import concourse.tile as tile_mod

    if getattr(tile_mod.TileContext, "_drain_patched", False):
        return

    def _patched(self, tick_clock, wait_clock):
        from concourse.vector_clock import ScopedClock

        vclock = tick_clock.global_clock
        pend = [(p, vclock[p]) for p in range(len(vclock)) if vclock[p] > 0]
        for proc, tick in pend:
            sub = ScopedClock()
            sub.require_at_least(None, proc, tick)
            nop_inst = self.nc.sync.nop(nofuse=True)
            wait_clock.add_sem_waits(nop_inst.ins, sub)
        self.nc.sync.drain()
        self.nc.all_engine_barrier()
        assert self.sems is not None
        popped = self.nc._tile_sem_poison_stack.pop()
        assert popped is self._sem_poison
        self.nc.clear_and_free_semaphores(list(self.sems.allocated().values()))
        self.nc.all_engine_barrier()

    tile_mod.TileContext._drain_and_barrier = _patched
    tile_mod.TileContext._drain_patched = True


def _legalize_sync_waits(nc, max_waits: int = 1):
    """walrus codegen here rejects instructions with >1 sem wait ("Too many
    sync wait commands"); hoist extra waits onto same-engine NoOps."""
    import concourse.mybir as mybir

    n = 0
    for f in nc.m.functions:
        for bb in f.blocks:
            out = []
            for ins in bb.instructions:
                si = ins.sync_info
                if si is not None and si.on_wait and len(si.on_wait) > max_waits:
                    waits = list(si.on_wait)
                    for w in waits[:-max_waits]:
                        n += 1
                        nop = mybir.InstNoOp(
                            name=f"waitnop_{n}", engine=ins.engine)
                        nop.sync_info = mybir.SyncInfo(
                            on_wait=[w], on_update=[])
                        out.append(nop)
                    si.on_wait = waits[-max_waits:]
                out.append(ins)
            bb.instructions = out


def _build(fast: bool):
    import concourse.bass as bass
    import concourse.mybir as mybir
    from concourse.tile import TileContext

    _apply_tile_drain_patch()

    fp16 = mybir.dt.float16
    fp32 = mybir.dt.float32
    AF = mybir.ActivationFunctionType
    OP = mybir.AluOpType

    nc = bass.Bass()
    xT_d = nc.dram_tensor("xT", [D, N, BC], fp16, kind="ExternalInput")
    kz_d = nc.dram_tensor("kz", [D, H], fp16, kind="ExternalInput")
    kr_d = nc.dram_tensor("kr", [D, H], fp16, kind="ExternalInput")
    kh_d = nc.dram_tensor("kh", [D, H], fp16, kind="ExternalInput")
    # per-hb epilogue bias columns [128, HB]: fast: XR bias = br-mr (=-1)
    brv_d = nc.dram_tensor("brv", [128, HB], fp32, kind="ExternalInput")
    if not fast:
        bzv_d = nc.dram_tensor("bzv", [128, HB], fp32, kind="ExternalInput")
        mrt_d = nc.dram_tensor("mrt", [128, P], fp16, kind="ExternalInput")
        mzt_d = nc.dram_tensor("mzt", [128, P], fp16, kind="ExternalInput")
    # ys stored [l, b, t, j, c] (h = (j*4+l)*128+c) so the post-transpose
    # chunk DMA is perfectly linear; host reassembles to [b, t, h].
    ys_d = nc.dram_tensor("ys", [HB // 2, BC, SEG, 2, 128], fp16,
                          kind="ExternalOutput")

    with TileContext(nc) as tc:
        with (
            tc.tile_pool(name="const", bufs=1) as cpool,
            tc.tile_pool(name="xk", bufs=2) as xkpool,
            tc.tile_pool(name="gates", bufs=3) as gpool,
            tc.tile_pool(name="scan", bufs=3) as spool,
            tc.tile_pool(name="ring", bufs=2) as rpool,
            tc.tile_pool(name="stg", bufs=2) as stpool,
            tc.tile_pool(name="psmm", bufs=6, space="PSUM") as pspool,
            tc.tile_pool(name="psmmh", bufs=2, space="PSUM") as pspoolh,
        ):
            # ---- weights / constants ----
            w_sb = {}
            for name, wd in (("z", kz_d), ("r", kr_d), ("h", kh_d)):
                wt = cpool.tile([128, KT * H], fp16, tag=f"w{name}")
                nc.sync.dma_start(
                    out=wt.rearrange("p (k h) -> p k h", k=KT),
                    in_=wd.rearrange("(k p) h -> p k h", p=128))
                for k in range(KT):
                    w_sb[(name, k)] = wt[:, k * H:(k + 1) * H]
            brv = cpool.tile([128, HB], fp32, tag="brv")
            nc.sync.dma_start(out=brv, in_=brv_d[:, :])
            if not fast:
                bzv = cpool.tile([128, HB], fp32, tag="bzv")
                nc.sync.dma_start(out=bzv, in_=bzv_d[:, :])
                mrt = cpool.tile([128, P], fp16, tag="mrt")
                nc.sync.dma_start(out=mrt, in_=mrt_d[:, :])
                mzt = cpool.tile([128, P], fp16, tag="mzt")
                nc.sync.dma_start(out=mzt, in_=mzt_d[:, :])

            hh0 = cpool.tile([128, P], fp16, tag="hh0")
            nc.vector.memset(hh0, 1.0)   # hh = h+1, h0 = 0
            hm0 = cpool.tile([128, P], fp16, tag="hm0")
            nc.vector.memset(hm0, 0.0)

            import bass_rust as _br

            _last = {}

            def _pin(eng, bi):
                # Pin each engine's stream to emission order; prevents
                # scheduler priority inversions (engines execute in-order).
                if eng in _last:
                    _br.add_dep_helper(bi.ins, _last[eng].ins, sync=False,
                                       reason=f"{eng} emission order")
                _last[eng] = bi
                return bi

            def vop(bi):
                return _pin("v", bi)

            def aop(bi):
                return _pin("a", bi)

            def gop(bi):
                return _pin("g", bi)

            def pe(bi):
                return _pin("pe", bi)

            # ---- GEMM pieces per chunk ----
            gates = {}   # ci -> (XR, XZ, XH) sbuf tiles [128, TC*P] fp16

            def make_pieces(ci):
                """Returns (loads, vps, aps, gps): closures for chunk ci's
                x loads and per-(gate,hb) matmul+epilogue groups, keyed by
                the epilogue engine."""
                XR = gpool.tile([128, TC * P], fp16, tag="XR", name=f"XR{ci}")
                XZ = gpool.tile([128, TC * P], fp16, tag="XZ", name=f"XZ{ci}")
                XH = gpool.tile([128, TC * P], fp16, tag="XH", name=f"XH{ci}")
                gates[ci] = (XR, XZ, XH)
                xk = [xkpool.tile([128, CB], fp16, tag=f"xk{k}",
                                  name=f"xk{k}_{ci}") for k in range(KT)]

                def load(k, xk=xk, ci=ci):
                    nc.sync.dma_start(
                        out=xk[k],
                        in_=xT_d[k * 128:(k + 1) * 128,
                                 ci * TC:(ci + 1) * TC, :])
                loads = [lambda k=k: load(k) for k in range(KT)]

                def mmgroup(g, hb, dest, ci=ci, xk=xk, lo=0, hi=TC):
                    nt = hi - lo
                    pool = pspool if nt == TC else pspoolh
                    ps = pool.tile([128, nt * BC], fp32,
                                   tag=("mm" if nt == TC else "mmh"),
                                   name=f"mm{ci}_{g}{hb}_{lo}")
                    for k in range(KT):
                        pe(nc.tensor.matmul(
                            out=ps,
                            lhsT=w_sb[(g, k)][:, hb * 128:(hb + 1) * 128],
                            rhs=xk[k][:, lo * BC:hi * BC],
                            start=(k == 0), stop=(k == KT - 1)))
                    dst = dest.rearrange(
                        "p (t hb b) -> p t hb b", t=TC, hb=HB)[:, lo:hi, hb, :]
                    ps3 = ps.rearrange("p (t b) -> p t b", t=nt)
                    # all epilogues on ACT: DVE is the throughput-bound
                    # engine, ACT has ~1us/step of idle after its sigmoids
                    if g == "r":      # XR: xr + (br - mr)
                        aop(nc.scalar.activation(
                            out=dst, in_=ps3, func=AF.Identity,
                            bias=brv[:, hb:hb + 1], scale=1.0))
                    elif g == "h":    # XH: xh / 2
                        aop(nc.scalar.activation(
                            out=dst, in_=ps3, func=AF.Identity,
                            bias=0.0, scale=0.5))
                    else:             # XZ: xz (+ bz)
                        aop(nc.scalar.activation(
                            out=dst, in_=ps3, func=AF.Identity,
                            bias=(0.0 if fast else bzv[:, hb:hb + 1]),
                            scale=1.0))

                eps = []
                if ci == 0:
                    # halve chunk 0's groups so the scan can start after
                    # only the first 8 steps' gates are ready
                    for lo, hi in ((0, TC // 2), (TC // 2, TC)):
                        for hb in range(HB):
                            for g, dd in (("r", XR), ("z", XZ), ("h", XH)):
                                eps.append(
                                    lambda g=g, hb=hb, dd=dd, lo=lo, hi=hi:
                                    mmgroup(g, hb, dd, lo=lo, hi=hi))
                else:
                    for hb in range(HB):
                        eps.append(lambda hb=hb: mmgroup("r", hb, XR))
                        eps.append(lambda hb=hb: mmgroup("z", hb, XZ))
                        eps.append(lambda hb=hb: mmgroup("h", hb, XH))
                return [loads, eps]

            def emit_output(ci, ring):
                """xbar-transpose chunk ci's hm ring and DMA to ys."""
                stg = stpool.tile([128, TC * P], fp16, tag="stg",
                                  name=f"stg{ci}")
                nc.sync.dma_start_transpose(
                    out=stg.rearrange("p (g m) -> p g m", m=128),
                    in_=ring.rearrange("p (g u) -> p g u", u=128))
                ot0 = ci * TC - W
                dst = ys_d[:, :, ot0:ot0 + TC, :, :].rearrange(
                    "l b t j c -> (l b) t j c")
                nc.sync.dma_start(
                    out=dst,
                    in_=stg.rearrange("p (t j c) -> p t j c", t=TC, j=2))

            # ---- emit: prime chunks 0 and 1, then scan with pipelining ----
            pieces = {0: make_pieces(0), 1: make_pieces(1)}
            for grp in pieces[0]:
                for p_ in grp:
                    p_()
            for grp in pieces[1]:
                for p_ in grp:
                    p_()

            hh, hm = hh0, hm0

            def s_tile(tag, i):
                return spool.tile([128, P], fp16, tag=tag, name=f"{tag}_{i}")

            def gate_col(gt, t):
                return gt[:, t * P:(t + 1) * P]

            # a_0 / c_0 (and general-path m*h temps)
            XR, XZ, XH = gates[0]
            a_t = s_tile("a", 0)
            c_t = s_tile("c", 0)
            if fast:
                vop(nc.vector.tensor_tensor(a_t, gate_col(XR, 0), hh, OP.add))
                gop(nc.gpsimd.tensor_tensor(c_t, gate_col(XZ, 0), hm, OP.add))
            else:
                t1 = s_tile("t1", 0)
                vop(nc.vector.tensor_tensor(t1, mrt, hm, OP.mult))
                vop(nc.vector.tensor_tensor(a_t, t1, gate_col(XR, 0), OP.add))
                t2 = s_tile("t2", 0)
                vop(nc.vector.tensor_tensor(t2, mzt, hm, OP.mult))
                vop(nc.vector.tensor_tensor(c_t, t2, gate_col(XZ, 0), OP.add))

            for ci in range(NCH):
                XR, XZ, XH = gates[ci]
                nxt = make_pieces(ci + 2) if ci + 2 < NCH else [[], []]
                loads, eps = nxt
                ring = (rpool.tile([128, TC * P], fp16, tag="ring",
                                   name=f"ring{ci}") if ci >= OC0 else None)
                # all of chunk ci+2's x loads must be emitted before any of
                # its matmul groups (else early groups read stale x tiles)
                while loads:
                    loads.pop(0)()
                for t in range(TC):
                    i = ci * TC + t
                    last = (i == N - 1)
                    # chain front: s, sh, e3, q  (a_t from previous tail)
                    s_ = s_tile("s", i)
                    aop(nc.scalar.activation(s_, a_t, AF.Sigmoid, scale=2.0))
                    sh = s_tile("sh", i)
                    vop(nc.vector.tensor_tensor(sh, hm, s_, OP.mult))
                    e3 = s_tile("e3", i)
                    vop(nc.vector.tensor_tensor(e3, sh, gate_col(XH, t),
                                                OP.add))
                    z_ = s_tile("z", i)
                    aop(nc.scalar.activation(z_, c_t, AF.Sigmoid))
                    q_ = s_tile("q", i)
                    aop(nc.scalar.activation(q_, e3, AF.Sigmoid, scale=4.0))
                    # 24 epilogue groups over 16 steps: 1 per step + 1 extra
                    # on odd steps, all in ACT's post-q idle window
                    if eps:
                        eps.pop(0)()
                    if t % 2 == 1 and eps:
                        eps.pop(0)()
                    # off-chain tail
                    U2 = s_tile("U2", i)
                    vop(nc.vector.tensor_scalar(
                        out=U2, in0=z_, scalar1=-2.0, scalar2=2.0,
                        op0=OP.mult, op1=OP.add))
                    hz1 = s_tile("hz1", i)
                    vop(nc.vector.tensor_tensor(hz1, hh, z_, OP.mult))
                    if fast and not last:
                        hz1x = s_tile("hz1x", i)
                        XRn = gates[ci + 1][0] if t == TC - 1 else XR
                        vop(nc.vector.tensor_tensor(
                            hz1x, hz1, gate_col(XRn, (t + 1) % TC), OP.add))
                    v_ = s_tile("vv", i)
                    vop(nc.vector.tensor_tensor(v_, q_, U2, OP.mult))
                    # chain-critical a' goes right after v; hh'/hm' have
                    # ~half a step of slack before their first consumers
                    if not last and fast:
                        a_t = s_tile("a", i + 1)
                        vop(nc.vector.tensor_tensor(a_t, v_, hz1x, OP.add))
                    hh_n = s_tile("hh", i)
                    vop(nc.vector.tensor_tensor(hh_n, v_, hz1, OP.add))
                    hm_n = (ring[:, t * P:(t + 1) * P] if ring is not None
                            else s_tile("hm", i))
                    vop(nc.vector.tensor_scalar(
                        out=hm_n, in0=hh_n, scalar1=-1.0, scalar2=None,
                        op0=OP.add))
                    if not last:
                        c_t = s_tile("c", i + 1)
                        if fast:
                            XZn = gates[ci + 1][1] if t == TC - 1 else XZ
                            vop(nc.vector.tensor_tensor(
                                c_t, gate_col(XZn, (t + 1) % TC), hm_n,
                                OP.add))
                        else:
                            a_t = s_tile("a", i + 1)
                            XRn = gates[ci + 1][0] if t == TC - 1 else XR
                            XZn = gates[ci + 1][1] if t == TC - 1 else XZ
                            t1 = s_tile("t1", i + 1)
                            vop(nc.vector.tensor_tensor(t1, mrt, hm_n,
                                                        OP.mult))
                            vop(nc.vector.tensor_tensor(
                                a_t, t1, gate_col(XRn, (t + 1) % TC), OP.add))
                            t2 = s_tile("t2", i + 1)
                            gop(nc.gpsimd.tensor_tensor(t2, mzt, hm_n,
                                                        OP.mult))
                            gop(nc.gpsimd.tensor_tensor(
                                c_t, t2, gate_col(XZn, (t + 1) % TC), OP.add))
                    hh = hh_n
                    hm = hm_n
                # drain leftover pieces, then output the chunk
                for grp in (loads, eps):
                    while grp:
                        grp.pop(0)()
                if ring is not None:
                    emit_output(ci, ring)

    _legalize_sync_waits(nc)
    return nc


def _get_nc(fast: bool):
    if fast not in _cache:
        _cache[fast] = _build(fast)
    return _cache[fast]


LAST_RESULT = None


def kernel(**inputs):
    global LAST_RESULT
    from concourse.bass_utils import run_bass_kernel_spmd

    x = np.asarray(inputs["x"], dtype=np.float32)
    kz = np.asarray(inputs["kz"], dtype=np.float32)
    kr = np.asarray(inputs["kr"], dtype=np.float32)
    kh = np.asarray(inputs["kh"], dtype=np.float32)
    mz = np.asarray(inputs["mz"], dtype=np.float32)
    mr = np.asarray(inputs["mr"], dtype=np.float32)
    br = np.asarray(inputs["br"], dtype=np.float32)
    bz = np.asarray(inputs["bz"], dtype=np.float32)
    assert x.shape == (B, T, D) and kz.shape == (D, H)

    fast = bool(np.all(mz == 1.0) and np.all(mr == 1.0))
    nc = _get_nc(fast)

    def pvec(v):  # [H] -> [128, HB] with [h_a, h_b]
        return np.ascontiguousarray(v.reshape(HB, 128).T)

    def ptile(v):  # [H] -> [128, (hb, b)] fp16, replicated over b
        t = v.reshape(HB, 128).T
        return np.ascontiguousarray(
            np.repeat(t[:, :, None], BC, axis=2).reshape(128, P)
        ).astype(np.float16)

    base = {
        "kz": np.ascontiguousarray(kz).astype(np.float16),
        "kr": np.ascontiguousarray(kr).astype(np.float16),
        "kh": np.ascontiguousarray(kh).astype(np.float16),
        "brv": pvec((br - mr) if fast else br).astype(np.float32),
    }
    if not fast:
        base["bzv"] = pvec(bz).astype(np.float32)
        base["mrt"] = ptile(mr)
        base["mzt"] = ptile(mz)

    x16 = x.astype(np.float16)
    in_maps = []
    for i in range(NCORES):
        i_t, i_b = i // SB, i % SB
        t0 = i_t * SEG
        bs = slice(i_b * BC, (i_b + 1) * BC)
        xc = np.zeros((BC, N, D), np.float16)
        src = x16[bs, max(0, t0 - W):t0 + SEG]
        xc[:, N - src.shape[1]:, :] = src
        xTc = np.ascontiguousarray(xc.transpose(2, 1, 0))
        in_maps.append(dict(base, xT=xTc))

    trace = bool(int(os.environ.get("KERNEL_TRACE", "0")))
    res = run_bass_kernel_spmd(nc, in_maps, list(range(NCORES)), trace=trace)
    LAST_RESULT = res
    ys = np.empty((B, T, H), np.float32)
    for i in range(NCORES):
        i_t, i_b = i // SB, i % SB
        yc = res.results[i]["ys"].astype(np.float32)  # [l, b, t, j, c]
        ys[i_b * BC:(i_b + 1) * BC, i_t * SEG:(i_t + 1) * SEG, :] = (
            yc.transpose(1, 2, 3, 0, 4).reshape(BC, SEG, H))
    return ys


# revision 41
# speedup vs baseline: 1.0941x; 1.0600x over previous
"""Trainium2 Bass kernel for nn_BRC_17179869451 (BRC-style RNN).

  xz/xr/xh = x @ {kz,kr,kh}   (three [B*T,D]x[D,H] GEMMs)
  scan over T:
      r = tanh(xr_t + h*mr + br) + 1
      z = sigmoid(xz_t + h*mz + bz)
      h = z*h + (1-z)*tanh(xh_t + r*h)

Sharding (8 cores = 4 time-segments x 2 batch-halves): the BRC forget
gate makes h_t depend only weakly on the distant past, so each core
computes a 128-step time segment for its 32-batch half, preceded by a
W=48-step redundant warmup from h=0 (validated offline: rel err ~2e-4
in fp64, ~2e-3 with the fp16 pipeline below).  Segment 0 zero-pads its
warmup input, which keeps h exactly 0.

Everything on-device runs fp16 (validated rel err ~1.9e-3 end to end):
fp16 GEMMs (1 PE pass instead of 4 for fp32), fp16 scan ops (DVE 2x/4x
perf modes), fp16 output staged via the xbar DMA-transpose and upcast
to fp32 on the host.

Per-step math (fast path mz=mr=1; hh = h+1 shifted state, hm = h):
  s = sigmoid(2*(xr-1 + hh))            r = 2s
  q = sigmoid(4*(hm*s + xh/2))          tanh(xh + r*h) = 2q-1
  z = sigmoid(xz + hm)
  hh' = 2q(1-z) + hh*z ;  ys = hm' = hh' - 1
as engine ops (V=DVE, A=ACT, G=gpsimd), gates pre-scaled in epilogue:
  chain: v=TT(q,U2) -> a+=TT(v,hz1x) -> s=ACT(a,sc2) -> sh=TT(hm,s)
         -> e3=TT(sh,XHH) -> q=ACT(e3,sc4)
  off:   hh+=TT(v,hz1); hm+=TS(hh+,-1); c+=G.TT(XZ0,hm+); z=ACT(c);
         U2=TS(z,-2,+2); hz1=TT(hh,z); hz1x=TT(hz1,XR1[t+1])
Layout per core: state [128 x 256]: partition h_a = h mod 128, free
(hb = h div 128 [8], b [32]).  Output: per 16-step chunk the hm ring
[128, (t,j,u)] is xbar-transposed to [u, (t,j), h_a] and DMA'd to
ys[b,t,h] in 256B runs.
"""

import os
import numpy as np

B, T, D, H = 64, 512, 512, 1024
NCORES = 8
ST = 4                    # time segments
SB = 2                    # batch shards
BC = B // SB              # 32 batches per core
SEG = T // ST             # 128 output steps per core
W = 32                    # warmup steps
N = SEG + W               # 176 steps computed per core
TC = 16                   # steps per chunk
NCH = N // TC             # 11 chunks
OC0 = W // TC             # first output chunk (3)
HB = H // 128             # 8 h-blocks
P = HB * BC               # 256 = free size of scan state
KT = D // 128             # 4 k-tiles
CB = TC * BC              # 512 matmul cols per chunk

_cache = {}


def _apply_tile_drain_patch():
    """Spread end-of-kernel sem waits over single-wait sync nops: walrus
    CoreV3 codegen rejects the stock Tile exit Drain that carries one wait
    per logical proc ("Too many sync wait commands")."""
    import concourse.tile as tile_mod

    if getattr(tile_mod.TileContext, "_drain_patched", False):
        return

    def _patched(self, tick_clock, wait_clock):
        from concourse.vector_clock import ScopedClock

        vclock = tick_clock.global_clock
        pend = [(p, vclock[p]) for p in range(len(vclock)) if vclock[p] > 0]
        for proc, tick in pend:
            sub = ScopedClock()
            sub.require_at_least(None, proc, tick)
            nop_inst = self.nc.sync.nop(nofuse=True)
            wait_clock.add_sem_waits(nop_inst.ins, sub)
        self.nc.sync.drain()
        self.nc.all_engine_barrier()
        assert self.sems is not None
        popped = self.nc._tile_sem_poison_stack.pop()
        assert popped is self._sem_poison
        self.nc.clear_and_free_semaphores(list(self.sems.allocated().values()))
        self.nc.all_engine_barrier()

    tile_mod.TileContext._drain_and_barrier = _patched
    tile_mod.TileContext._drain_patched = True


def _legalize_sync_waits(nc, max_waits: int = 1):
    """walrus codegen here rejects instructions with >1 sem wait ("Too many
    sync wait commands"); hoist extra waits onto same-engine NoOps."""
    import concourse.mybir as mybir

    n = 0
    for f in nc.m.functions:
        for bb in f.blocks:
            out = []
            for ins in bb.instructions:
                si = ins.sync_info
                if si is not None and si.on_wait and len(si.on_wait) > max_waits:
                    waits = list(si.on_wait)
                    for w in waits[:-max_waits]:
                        n += 1
                        nop = mybir.InstNoOp(
                            name=f"waitnop_{n}", engine=ins.engine)
                        nop.sync_info = mybir.SyncInfo(
                            on_wait=[w], on_update=[])
                        out.append(nop)
                    si.on_wait = waits[-max_waits:]
                out.append(ins)
            bb.instructions = out


def _build(fast: bool):
    import concourse.bass as bass
    import concourse.mybir as mybir
    from concourse.tile import TileContext

    _apply_tile_drain_patch()

    fp16 = mybir.dt.float16
    fp32 = mybir.dt.float32
    AF = mybir.ActivationFunctionType
    OP = mybir.AluOpType

    nc = bass.Bass()
    xT_d = nc.dram_tensor("xT", [D, N, BC], fp16, kind="ExternalInput")
    kz_d = nc.dram_tensor("kz", [D, H], fp16, kind="ExternalInput")
    kr_d = nc.dram_tensor("kr", [D, H], fp16, kind="ExternalInput")
    kh_d = nc.dram_tensor("kh", [D, H], fp16, kind="ExternalInput")
    # per-hb epilogue bias columns [128, HB]: fast: XR bias = br-mr (=-1)
    brv_d = nc.dram_tensor("brv", [128, HB], fp32, kind="ExternalInput")
    if not fast:
        bzv_d = nc.dram_tensor("bzv", [128, HB], fp32, kind="ExternalInput")
        mrt_d = nc.dram_tensor("mrt", [128, P], fp16, kind="ExternalInput")
        mzt_d = nc.dram_tensor("mzt", [128, P], fp16, kind="ExternalInput")
    # ys stored [l, b, t, j, c] (h = (j*4+l)*128+c) so the post-transpose
    # chunk DMA is perfectly linear; host reassembles to [b, t, h].
    ys_d = nc.dram_tensor("ys", [HB // 2, BC, SEG, 2, 128], fp16,
                          kind="ExternalOutput")

    with TileContext(nc) as tc:
        with (
            tc.tile_pool(name="const", bufs=1) as cpool,
            tc.tile_pool(name="xk", bufs=2) as xkpool,
            tc.tile_pool(name="gates", bufs=3) as gpool,
            tc.tile_pool(name="scan", bufs=3) as spool,
            tc.tile_pool(name="ring", bufs=2) as rpool,
            tc.tile_pool(name="stg", bufs=2) as stpool,
            tc.tile_pool(name="psmm", bufs=6, space="PSUM") as pspool,
            tc.tile_pool(name="psmmh", bufs=2, space="PSUM") as pspoolh,
        ):
            # ---- weights / constants ----
            w_sb = {}
            for name, wd in (("z", kz_d), ("r", kr_d), ("h", kh_d)):
                wt = cpool.tile([128, KT * H], fp16, tag=f"w{name}")
                nc.sync.dma_start(
                    out=wt.rearrange("p (k h) -> p k h", k=KT),
                    in_=wd.rearrange("(k p) h -> p k h", p=128))
                for k in range(KT):
                    w_sb[(name, k)] = wt[:, k * H:(k + 1) * H]
            brv = cpool.tile([128, HB], fp32, tag="brv")
            nc.sync.dma_start(out=brv, in_=brv_d[:, :])
            if not fast:
                bzv = cpool.tile([128, HB], fp32, tag="bzv")
                nc.sync.dma_start(out=bzv, in_=bzv_d[:, :])
                mrt = cpool.tile([128, P], fp16, tag="mrt")
                nc.sync.dma_start(out=mrt, in_=mrt_d[:, :])
                mzt = cpool.tile([128, P], fp16, tag="mzt")
                nc.sync.dma_start(out=mzt, in_=mzt_d[:, :])

            hh0 = cpool.tile([128, P], fp16, tag="hh0")
            nc.vector.memset(hh0, 1.0)   # hh = h+1, h0 = 0
            hm0 = cpool.tile([128, P], fp16, tag="hm0")
            nc.vector.memset(hm0, 0.0)

            import bass_rust as _br

            _last = {}

            def _pin(eng, bi):
                # Pin each engine's stream to emission order; prevents
                # scheduler priority inversions (engines execute in-order).
                if eng in _last:
                    _br.add_dep_helper(bi.ins, _last[eng].ins, sync=False,
                                       reason=f"{eng} emission order")
                _last[eng] = bi
                return bi

            def vop(bi):
                return _pin("v", bi)

            def aop(bi):
                return _pin("a", bi)

            def gop(bi):
                return _pin("g", bi)

            def pe(bi):
                return _pin("pe", bi)

            # ---- GEMM pieces per chunk ----
            gates = {}   # ci -> (XR, XZ, XH) sbuf tiles [128, TC*P] fp16

            def make_pieces(ci):
                """Returns (loads, vps, aps, gps): closures for chunk ci's
                x loads and per-(gate,hb) matmul+epilogue groups, keyed by
                the epilogue engine."""
                XR = gpool.tile([128, TC * P], fp16, tag="XR", name=f"XR{ci}")
                XZ = gpool.tile([128, TC * P], fp16, tag="XZ", name=f"XZ{ci}")
                XH = gpool.tile([128, TC * P], fp16, tag="XH", name=f"XH{ci}")
                gates[ci] = (XR, XZ, XH)
                xk = [xkpool.tile([128, CB], fp16, tag=f"xk{k}",
                                  name=f"xk{k}_{ci}") for k in range(KT)]

                def load(k, xk=xk, ci=ci):
                    nc.sync.dma_start(
                        out=xk[k],
                        in_=xT_d[k * 128:(k + 1) * 128,
                                 ci * TC:(ci + 1) * TC, :])
                loads = [lambda k=k: load(k) for k in range(KT)]

                def mmgroup(g, hb, dest, ci=ci, xk=xk, lo=0, hi=TC):
                    nt = hi - lo
                    pool = pspool if nt == TC else pspoolh
                    ps = pool.tile([128, nt * BC], fp32,
                                   tag=("mm" if nt == TC else "mmh"),
                                   name=f"mm{ci}_{g}{hb}_{lo}")
                    for k in range(KT):
                        pe(nc.tensor.matmul(
                            out=ps,
                            lhsT=w_sb[(g, k)][:, hb * 128:(hb + 1) * 128],
                            rhs=xk[k][:, lo * BC:hi * BC],
                            start=(k == 0), stop=(k == KT - 1)))
                    dst = dest.rearrange(
                        "p (t hb b) -> p t hb b", t=TC, hb=HB)[:, lo:hi, hb, :]
                    ps3 = ps.rearrange("p (t b) -> p t b", t=nt)
                    # all epilogues on ACT: DVE is the throughput-bound
                    # engine, ACT has ~1us/step of idle after its sigmoids
                    if g == "r":      # XR: xr + (br - mr)
                        aop(nc.scalar.activation(
                            out=dst, in_=ps3, func=AF.Identity,
                            bias=brv[:, hb:hb + 1], scale=1.0))
                    elif g == "h":    # XH: xh / 2
                        aop(nc.scalar.activation(
                            out=dst, in_=ps3, func=AF.Identity,
                            bias=0.0, scale=0.5))
                    else:             # XZ: xz (+ bz)
                        aop(nc.scalar.activation(
                            out=dst, in_=ps3, func=AF.Identity,
                            bias=(0.0 if fast else bzv[:, hb:hb + 1]),
                            scale=1.0))

                eps = []
                for hb in range(HB):
                    eps.append(lambda hb=hb: mmgroup("r", hb, XR))
                    eps.append(lambda hb=hb: mmgroup("z", hb, XZ))
                    eps.append(lambda hb=hb: mmgroup("h", hb, XH))
                return [loads, eps]

            def emit_output(ci, ring):
                """xbar-transpose chunk ci's hm ring and DMA to ys."""
                stg = stpool.tile([128, TC * P], fp16, tag="stg",
                                  name=f"stg{ci}")
                nc.sync.dma_start_transpose(
                    out=stg.rearrange("p (g m) -> p g m", m=128),
                    in_=ring.rearrange("p (g u) -> p g u", u=128))
                ot0 = ci * TC - W
                dst = ys_d[:, :, ot0:ot0 + TC, :, :].rearrange(
                    "l b t j c -> (l b) t j c")
                nc.sync.dma_start(
                    out=dst,
                    in_=stg.rearrange("p (t j c) -> p t j c", t=TC, j=2))

            # ---- emit: prime only chunk 0 (chunk 1+ interleave into the
            # scan steps — priming more would queue their epilogues ahead
            # of the first sigmoids in ACT's in-order stream) ----
            for grp in make_pieces(0):
                for p_ in grp:
                    p_()

            hh, hm = hh0, hm0

            def s_tile(tag, i):
                return spool.tile([128, P], fp16, tag=tag, name=f"{tag}_{i}")

            def gate_col(gt, t):
                return gt[:, t * P:(t + 1) * P]

            # a_0 / c_0 (and general-path m*h temps)
            XR, XZ, XH = gates[0]
            a_t = s_tile("a", 0)
            c_t = s_tile("c", 0)
            if fast:
                vop(nc.vector.tensor_tensor(a_t, gate_col(XR, 0), hh, OP.add))
                gop(nc.gpsimd.tensor_tensor(c_t, gate_col(XZ, 0), hm, OP.add))
            else:
                t1 = s_tile("t1", 0)
                vop(nc.vector.tensor_tensor(t1, mrt, hm, OP.mult))
                vop(nc.vector.tensor_tensor(a_t, t1, gate_col(XR, 0), OP.add))
                t2 = s_tile("t2", 0)
                vop(nc.vector.tensor_tensor(t2, mzt, hm, OP.mult))
                vop(nc.vector.tensor_tensor(c_t, t2, gate_col(XZ, 0), OP.add))

            for ci in range(NCH):
                XR, XZ, XH = gates[ci]
                nxt = make_pieces(ci + 1) if ci + 1 < NCH else [[], []]
                loads, eps = nxt
                ring = (rpool.tile([128, TC * P], fp16, tag="ring",
                                   name=f"ring{ci}") if ci >= OC0 else None)
                # all of chunk ci+2's x loads must be emitted before any of
                # its matmul groups (else early groups read stale x tiles)
                while loads:
                    loads.pop(0)()
                for t in range(TC):
                    i = ci * TC + t
                    last = (i == N - 1)
                    # chain front: s, sh, e3, q  (a_t from previous tail)
                    s_ = s_tile("s", i)
                    aop(nc.scalar.activation(s_, a_t, AF.Sigmoid, scale=2.0))
                    sh = s_tile("sh", i)
                    vop(nc.vector.tensor_tensor(sh, hm, s_, OP.mult))
                    e3 = s_tile("e3", i)
                    vop(nc.vector.tensor_tensor(e3, sh, gate_col(XH, t),
                                                OP.add))
                    z_ = s_tile("z", i)
                    aop(nc.scalar.activation(z_, c_t, AF.Sigmoid))
                    q_ = s_tile("q", i)
                    aop(nc.scalar.activation(q_, e3, AF.Sigmoid, scale=4.0))
                    # 24 epilogue groups over 16 steps: 1 per step + 1 extra
                    # on odd steps, all in ACT's post-q idle window
                    if eps:
                        eps.pop(0)()
                    if t % 2 == 1 and eps:
                        eps.pop(0)()
                    # off-chain tail
                    U2 = s_tile("U2", i)
                    vop(nc.vector.tensor_scalar(
                        out=U2, in0=z_, scalar1=-2.0, scalar2=2.0,
                        op0=OP.mult, op1=OP.add))
                    hz1 = s_tile("hz1", i)
                    vop(nc.vector.tensor_tensor(hz1, hh, z_, OP.mult))
                    if fast and not last:
                        hz1x = s_tile("hz1x", i)
                        XRn = gates[ci + 1][0] if t == TC - 1 else XR
                        vop(nc.vector.tensor_tensor(
                            hz1x, hz1, gate_col(XRn, (t + 1) % TC), OP.add))
                    v_ = s_tile("vv", i)
                    vop(nc.vector.tensor_tensor(v_, q_, U2, OP.mult))
                    # chain-critical a' goes right after v; hh'/hm' have
                    # ~half a step of slack before their first consumers
                    if not last and fast:
                        a_t = s_tile("a", i + 1)
                        vop(nc.vector.tensor_tensor(a_t, v_, hz1x, OP.add))
                    hh_n = s_tile("hh", i)
                    vop(nc.vector.tensor_tensor(hh_n, v_, hz1, OP.add))
                    hm_n = (ring[:, t * P:(t + 1) * P] if ring is not None
                            else s_tile("hm", i))
                    vop(nc.vector.tensor_scalar(
                        out=hm_n, in0=hh_n, scalar1=-1.0, scalar2=None,
                        op0=OP.add))
                    if not last:
                        c_t = s_tile("c", i + 1)
                        if fast:
                            XZn = gates[ci + 1][1] if t == TC - 1 else XZ
                            vop(nc.vector.tensor_tensor(
                                c_t, gate_col(XZn, (t + 1) % TC), hm_n,
                                OP.add))
                        else:
                            a_t = s_tile("a", i + 1)
                            XRn = gates[ci + 1][0] if t == TC - 1 else XR
                            XZn = gates[ci + 1][1] if t == TC - 1 else XZ
                            t1 = s_tile("t1", i + 1)
                            vop(nc.vector.tensor_tensor(t1, mrt, hm_n,
                                                        OP.mult))
                            vop(nc.vector.tensor_tensor(
                                a_t, t1, gate_col(XRn, (t + 1) % TC), OP.add))
                            t2 = s_tile("t2", i + 1)
                            gop(nc.gpsimd.tensor_tensor(t2, mzt, hm_n,
                                                        OP.mult))
                            gop(nc.gpsimd.tensor_tensor(
                                c_t, t2, gate_col(XZn, (t + 1) % TC), OP.add))
                    hh = hh_n
                    hm = hm_n
                # drain leftover pieces, then output the chunk
                for grp in (loads, eps):
                    while grp:
                        grp.pop(0)()
                if ring is not None:
                    emit_output(ci, ring)

    _legalize_sync_waits(nc)
    return nc


def _get_nc(fast: bool):
    if fast not in _cache:
        _cache[fast] = _build(fast)
    return _cache[fast]


LAST_RESULT = None


def kernel(**inputs):
    global LAST_RESULT
    from concourse.bass_utils import run_bass_kernel_spmd

    x = np.asarray(inputs["x"], dtype=np.float32)
    kz = np.asarray(inputs["kz"], dtype=np.float32)
    kr = np.asarray(inputs["kr"], dtype=np.float32)
    kh = np.asarray(inputs["kh"], dtype=np.float32)
    mz = np.asarray(inputs["mz"], dtype=np.float32)
    mr = np.asarray(inputs["mr"], dtype=np.float32)
    br = np.asarray(inputs["br"], dtype=np.float32)
    bz = np.asarray(inputs["bz"], dtype=np.float32)
    assert x.shape == (B, T, D) and kz.shape == (D, H)

    fast = bool(np.all(mz == 1.0) and np.all(mr == 1.0))
    nc = _get_nc(fast)

    def pvec(v):  # [H] -> [128, HB] with [h_a, h_b]
        return np.ascontiguousarray(v.reshape(HB, 128).T)

    def ptile(v):  # [H] -> [128, (hb, b)] fp16, replicated over b
        t = v.reshape(HB, 128).T
        return np.ascontiguousarray(
            np.repeat(t[:, :, None], BC, axis=2).reshape(128, P)
        ).astype(np.float16)

    base = {
        "kz": np.ascontiguousarray(kz).astype(np.float16),
        "kr": np.ascontiguousarray(kr).astype(np.float16),
        "kh": np.ascontiguousarray(kh).astype(np.float16),
        "brv": pvec((br - mr) if fast else br).astype(np.float32),
    }
    if not fast:
        base["bzv"] = pvec(bz).astype(np.float32)
        base["mrt"] = ptile(mr)
        base["mzt"] = ptile(mz)

    x16 = x.astype(np.float16)
    in_maps = []
    for i in range(NCORES):
        i_t, i_b = i // SB, i % SB
        t0 = i_t * SEG
        bs = slice(i_b * BC, (i_b + 1) * BC)
        xc = np.zeros((BC, N, D), np.float16)
        src = x16[bs, max(0, t0 - W):t0 + SEG]
        xc[:, N - src.shape[1]:, :] = src
        xTc = np.ascontiguousarray(xc.transpose(2, 1, 0))
        in_maps.append(dict(base, xT=xTc))

    trace = bool(int(os.environ.get("KERNEL_TRACE", "0")))
    res = run_bass_kernel_spmd(nc, in_maps, list(range(NCORES)), trace=trace)
    LAST_RESULT = res
    ys = np.empty((B, T, H), np.float32)
    for i in range(NCORES):
        i_t, i_b = i // SB, i % SB
        yc = res.results[i]["ys"].astype(np.float32)  # [l, b, t, j, c]
        ys[i_b * BC:(i_b + 1) * BC, i_t * SEG:(i_t + 1) * SEG, :] = (
            yc.transpose(1, 2, 3, 0, 4).reshape(BC, SEG, H))
    return ys
